# revision 8
# baseline (speedup 1.0000x reference)
"""DeepSeek-V2 MoE grouped-GEMM expert FFN (SwiGLU) on 8 Trainium2 NeuronCores.

Expert-parallel: tokens are pre-sorted by expert; each core gets a set of
(expert weights, <=512-token tile) work items. All three GEMMs keep the
weights as the stationary (lhsT) operand and stream activations token-major:

  gate^T[n,tok] = sum_k  gate_w[k,n]^T @ x^T[k,tok]     (k over HIDDEN/128)
  act  = silu(gate^T) * up^T        (bf16)
  y^T[h,tok]   = sum_f  down_w[f,h]^T @ act[f,tok]      (f over INTER/128)

Weights are host-rearranged per (tile, out-block) into [128, nk*128] slabs so
every weight DMA is a single large linear transfer and the device consumes
weights in exactly streaming order (each weight element is used once).
Compute dtype bf16, accumulation fp32 in PSUM, output fp32.
"""

import sys

if "/opt/trn_rl_repo" not in sys.path:
    sys.path.insert(0, "/opt/trn_rl_repo")

import numpy as np
import ml_dtypes

N_CORES = 8
HIDDEN = 2048
INTER = 1408
TOK_TILE = 512
KT = HIDDEN // 128  # 16
FT = INTER // 128   # 11

_NC_CACHE = {}


def _build_nc(T):
    """Bass program for one core: T independent (weights, 512-token) work items."""
    import concourse.bacc as bacc
    import concourse.mybir as mybir
    import concourse.tile as tile

    bf16 = mybir.dt.bfloat16
    f32 = mybir.dt.float32

    nc = bacc.Bacc("TRN2", target_bir_lowering=False, debug=False)
    xt = nc.dram_tensor("xt", [T, KT, 128, TOK_TILE], bf16, kind="ExternalInput")
    gw = nc.dram_tensor("gw", [T, FT, 128, HIDDEN], bf16, kind="ExternalInput")
    uw = nc.dram_tensor("uw", [T, FT, 128, HIDDEN], bf16, kind="ExternalInput")
    dw = nc.dram_tensor("dw", [T, KT, 128, INTER], bf16, kind="ExternalInput")
    yt = nc.dram_tensor("yt", [T, KT, 128, TOK_TILE], f32, kind="ExternalOutput")

    with tile.TileContext(nc) as tc:
        with (
            tc.tile_pool(name="xpool", bufs=2 * KT) as xpool,
            tc.tile_pool(name="wpool", bufs=5) as wpool,
            tc.tile_pool(name="apool", bufs=2 * FT) as apool,
            tc.tile_pool(name="spool", bufs=3) as spool,
            tc.tile_pool(name="opool", bufs=4) as opool,
            tc.tile_pool(name="psA", bufs=2, space="PSUM") as psA,
            tc.tile_pool(name="psB", bufs=3, space="PSUM") as psB,
        ):
            for t in range(T):
                # n=0 weight tiles are on the critical path at kernel start:
                # issue them (chunked, so the k=0 matmul only waits on the
                # first 128-column slab) before the x loads on the same ring.
                gw0 = wpool.tile([128, HIDDEN], bf16, name=f"gw_{t}_0", tag="gw")
                uw0 = wpool.tile([128, HIDDEN], bf16, name=f"uw_{t}_0", tag="uw")
                nchunk = 4 if t == 0 else 1
                cw = HIDDEN // nchunk
                for ci in range(nchunk):
                    sl = slice(ci * cw, (ci + 1) * cw)
                    nc.sync.dma_start(gw0[:, sl], gw[t, 0, :, sl])
                    nc.sync.dma_start(uw0[:, sl], uw[t, 0, :, sl])

                # everything loads on sync's ring in exact consume order; deep
                # buffer pools keep slot-waits from head-of-line blocking it
                xts = []
                for k in range(KT):
                    xk = xpool.tile([128, TOK_TILE], bf16, name=f"x_{t}_{k}", tag="x",
                                    bufs=40)
                    nc.sync.dma_start(xk[:], xt[t, k, :, :])
                    xts.append(xk)

                acts = []
                for n in range(FT):
                    if n == 0:
                        gwt, uwt = gw0, uw0
                    else:
                        gwt = wpool.tile([128, HIDDEN], bf16, name=f"gw_{t}_{n}", tag="gw")
                        nc.sync.dma_start(gwt[:], gw[t, n, :, :])
                        uwt = wpool.tile([128, HIDDEN], bf16, name=f"uw_{t}_{n}", tag="uw")
                        nc.sync.dma_start(uwt[:], uw[t, n, :, :])

                    psg = psA.tile([128, TOK_TILE], f32, name=f"psg_{t}_{n}", tag="psg")
                    psu = psA.tile([128, TOK_TILE], f32, name=f"psu_{t}_{n}", tag="psu")
                    for k in range(KT):
                        nc.tensor.matmul(
                            psg[:], gwt[:, k * 128:(k + 1) * 128], xts[k][:],
                            start=(k == 0), stop=(k == KT - 1),
                        )
                    for k in range(KT):
                        nc.tensor.matmul(
                            psu[:], uwt[:, k * 128:(k + 1) * 128], xts[k][:],
                            start=(k == 0), stop=(k == KT - 1),
                        )

                    sg = spool.tile([128, TOK_TILE], f32, name=f"sg_{t}_{n}", tag="sg")
                    nc.scalar.activation(
                        sg[:], psg[:], mybir.ActivationFunctionType.Silu
                    )
                    at = apool.tile([128, TOK_TILE], bf16, name=f"act_{t}_{n}", tag="act")
                    nc.vector.tensor_mul(at[:], sg[:], psu[:])
                    acts.append(at)

                for h in range(KT):
                    dwt = wpool.tile([128, INTER], bf16, name=f"dw_{t}_{h}", tag="dw",
                                     bufs=6)
                    nc.sync.dma_start(dwt[:], dw[t, h, :, :])
                    psy = psB.tile([128, TOK_TILE], f32, name=f"psy_{t}_{h}", tag="psy")
                    for f in range(FT):
                        nc.tensor.matmul(
                            psy[:], dwt[:, f * 128:(f + 1) * 128], acts[f][:],
                            start=(f == 0), stop=(f == FT - 1),
                        )
                    ot = opool.tile([128, TOK_TILE], f32, name=f"o_{t}_{h}", tag="o")
                    # stores ride the ACT engine's HWDGE ring so they never
                    # head-of-line block the load prefetch stream on sync's ring.
                    # The very last store is split so copy/DMA/receipt overlap
                    # instead of serializing after the final matmul.
                    if t == T - 1 and h == KT - 1:
                        half = TOK_TILE // 2
                        for ci in range(2):
                            sl = slice(ci * half, (ci + 1) * half)
                            nc.vector.tensor_copy(ot[:, sl], psy[:, sl])
                            nc.scalar.dma_start(yt[t, h, :, sl], ot[:, sl])
                    else:
                        nc.vector.tensor_copy(ot[:], psy[:])
                        nc.scalar.dma_start(yt[t, h, :, :], ot[:])

    nc.compile()
    return nc


def _get_nc(T):
    if T not in _NC_CACHE:
        _NC_CACHE[T] = _build_nc(T)
    return _NC_CACHE[T]


def kernel(hidden_states, gate_w, up_w, down_w, group_sizes):
    from concourse.bass_utils import run_bass_kernel_spmd

    bf16 = ml_dtypes.bfloat16
    X = np.ascontiguousarray(np.asarray(hidden_states))
    gs = np.asarray(group_sizes).astype(np.int64)
    num_tokens, H = X.shape
    E, _, F = gate_w.shape
    assert H == HIDDEN and F == INTER

    # work-item list: (expert, row_start, nrows), rows grouped by expert
    tiles = []
    off = 0
    for e in range(E):
        m = int(gs[e])
        s = 0
        while s < m:
            nr = min(TOK_TILE, m - s)
            tiles.append((e, off + s, nr))
            s += nr
        off += m

    out = np.zeros((num_tokens, H), dtype=np.float32)
    if not tiles:
        return out
    while len(tiles) % N_CORES:
        tiles.append((tiles[0][0], 0, 0))  # dummy pad tile; output discarded
    T = len(tiles) // N_CORES

    Xb = X.astype(bf16)
    Gb = np.asarray(gate_w).astype(bf16)
    Ub = np.asarray(up_w).astype(bf16)
    Db = np.asarray(down_w).astype(bf16)

    # per-expert weight rearrangement (cached per expert within this call)
    g_cache, u_cache, d_cache = {}, {}, {}

    def g_r(e):
        if e not in g_cache:
            g_cache[e] = np.ascontiguousarray(
                Gb[e].reshape(KT, 128, FT, 128).transpose(2, 1, 0, 3)
            ).reshape(FT, 128, HIDDEN)
        return g_cache[e]

    def u_r(e):
        if e not in u_cache:
            u_cache[e] = np.ascontiguousarray(
                Ub[e].reshape(KT, 128, FT, 128).transpose(2, 1, 0, 3)
            ).reshape(FT, 128, HIDDEN)
        return u_cache[e]

    def d_r(e):
        if e not in d_cache:
            d_cache[e] = np.ascontiguousarray(
                Db[e].reshape(FT, 128, KT, 128).transpose(2, 1, 0, 3)
            ).reshape(KT, 128, INTER)
        return d_cache[e]

    in_maps = []
    for c in range(N_CORES):
        tl = tiles[c * T:(c + 1) * T]
        xt = np.zeros((T, KT, 128, TOK_TILE), dtype=bf16)
        gw = np.empty((T, FT, 128, HIDDEN), dtype=bf16)
        uw = np.empty((T, FT, 128, HIDDEN), dtype=bf16)
        dw = np.empty((T, KT, 128, INTER), dtype=bf16)
        for i, (e, r0, nr) in enumerate(tl):
            if nr:
                xt[i, :, :, :nr] = Xb[r0:r0 + nr].T.reshape(KT, 128, nr)
            gw[i] = g_r(e)
            uw[i] = u_r(e)
            dw[i] = d_r(e)
        in_maps.append({"xt": xt, "gw": gw, "uw": uw, "dw": dw})

    nc = _get_nc(T)
    res = run_bass_kernel_spmd(nc, in_maps, core_ids=list(range(N_CORES)))

    for c in range(N_CORES):
        ytc = res.results[c]["yt"]  # [T, KT, 128, TOK_TILE] f32
        for i, (e, r0, nr) in enumerate(tiles[c * T:(c + 1) * T]):
            if nr:
                out[r0:r0 + nr] = (
                    ytc[i].transpose(2, 0, 1).reshape(TOK_TILE, H)[:nr]
                )
    return out
